# revision 35
# baseline (speedup 1.0000x reference)
"""Multi-head attention (RoPE, causal) Trainium2 Bass kernel, 8-way sharded.

Sharding: core c handles batch b = c//4 and heads 4*(c%4)..4*(c%4)+3
(B*H = 32 head-rows -> 4 per core).  QKV/out projections are
Megatron-sliced per core; per-core partial outputs (row-parallel Wo)
are summed on the host.

v2 schedule: the attention inner loop is software-pipelined (scores of
step jj issue before the AV of step jj-2, so the PE never queues behind
the scalar-engine exp), and the projections / Wo / V-projection of
neighbouring chunks are drip-fed as PE filler inside the exp-bound
attention phase.  Normalization gathers softmax denominators via
SBUF->SBUF DMA into a squat [128,8] tile (parallel-lane reciprocal),
broadcasts the reciprocal back with a K=1 matmul.  Output is stored
bf16 and summed on the host in fp32.

Problem constants (hardcoded per contract):
  B=2, S=2048, D=1024, H=16, DK=64
"""

import math

import ml_dtypes
import numpy as np

import concourse.bass as bass
import concourse.mybir as mybir
import concourse.tile as tile
from concourse import bacc
from concourse.bass_utils import run_bass_kernel_spmd

B, S, D, H, DK = 2, 2048, 1024, 16, 64
E = 256            # head dims per core (4 heads x 64)
CH = 512           # sequence chunk (matmul free dim)
NCH = S // CH      # 4
NST = S // 128     # 16 s-tiles
BF16 = mybir.dt.bfloat16
F32 = mybir.dt.float32


def _np_reference_fallback(q, k, v, mask, Wq, bq, Wk, bk, Wv, bv, Wo, bo):
    """Pure-numpy reference path (only used for inputs outside the
    contract: non-causal mask or nonzero qkv biases)."""
    qh = (q @ Wq.T + bq).reshape(B, S, H, DK)
    kh = (k @ Wk.T + bk).reshape(B, S, H, DK)
    vh = (v @ Wv.T + bv).reshape(B, S, H, DK)
    inv_freq = 1.0 / (10000.0 ** (np.arange(0, DK, 2, dtype=np.float32) / DK))
    pos = np.arange(S, dtype=np.float32)
    fr = pos[:, None] * inv_freq[None, :]
    cos, sin = np.cos(fr)[:, None, :], np.sin(fr)[:, None, :]

    def rope(x):
        t = DK // 2
        x1, x2 = x[..., :t], x[..., t:]
        return np.concatenate([x1 * cos - x2 * sin, x1 * sin + x2 * cos], -1)

    qh, kh = rope(qh), rope(kh)
    sc = np.einsum('bqhd,bkhd->bhqk', qh, kh) / math.sqrt(DK)
    sc = np.where(mask == 0, np.float32(-10000.0), sc)
    sc = sc - sc.max(-1, keepdims=True)
    e = np.exp(sc)
    attn = e / e.sum(-1, keepdims=True)
    out = np.einsum('bhqk,bkhd->bqhd', attn, vh).reshape(B, S, D)
    return (out @ Wo.T + bo).astype(np.float32)


def _build_program():
    nc = bacc.Bacc(None, target_bir_lowering=False)

    dp = nc.declare_dram_parameter
    xq = dp("xq", [D, S], BF16, isOutput=False)   # q[b].T
    xk = dp("xk", [D, S], BF16, isOutput=False)
    xv = dp("xv", [D, S], BF16, isOutput=False)
    wq = dp("wq", [D, E], BF16, isOutput=False)   # Wq_c.T
    wk = dp("wk", [D, E], BF16, isOutput=False)
    wv = dp("wv", [D, E], BF16, isOutput=False)
    wo = dp("wo", [E, D], BF16, isOutput=False)   # Wo_c.T rows
    ct = dp("ct", [E, S], BF16, isOutput=False)   # cos table (1/sqrt8 folded)
    st = dp("st", [E, S], BF16, isOutput=False)   # signed sin table
    rt = dp("rt", [128, 128], BF16, isOutput=False)  # half-swap permutation
    tri = dp("tri", [128, 128], BF16, isOutput=False)  # causal 0/1 triangle
    out = dp("out", [S, D], BF16, isOutput=True)

    with tile.TileContext(nc) as tc:
        with (
            tc.tile_pool(name="const", bufs=1) as const,
            tc.tile_pool(name="persist", bufs=1) as persist,
            tc.tile_pool(name="xt", bufs=6) as xtp,
            tc.tile_pool(name="raw", bufs=4) as rawp,
            tc.tile_pool(name="ropetmp", bufs=4) as rtp,
            tc.tile_pool(name="pblk", bufs=4) as pblk,
            tc.tile_pool(name="norm", bufs=8) as normp,
            tc.tile_pool(name="obuf", bufs=4) as obufp,
            tc.tile_pool(name="projps", bufs=2, space="PSUM") as projps,
        ):
            # ---- constants to SBUF ----
            warm_t = const.tile([128, 128], BF16, tag="warm")
            nc.vector.memset(warm_t[:], 0.25)
            rt_t = const.tile([128, 128], BF16, tag="rt")
            wk_t = const.tile([128, 8, E], BF16, tag="wk")
            wq_t = const.tile([128, 8, E], BF16, tag="wq")
            ct_t = const.tile([128, 2, S], BF16, tag="ct")
            st_t = const.tile([128, 2, S], BF16, tag="st")
            wv_t = const.tile([128, 8, E], BF16, tag="wv")
            tri_t = const.tile([128, 128], BF16, tag="tri")
            wo_t = const.tile([128, 2, D], BF16, tag="wo")
            ones_t = const.tile([1, 64], F32, tag="ones")
            nc.vector.memset(ones_t[:], 1.0)

            # ---- persistent intermediates ----
            qT = persist.tile([128, 2, S], BF16, tag="qT")   # partitions: e%128, dim1: e//128
            kT = persist.tile([128, 2, S], BF16, tag="kT")
            aT = persist.tile([128, 2, S], BF16, tag="aT")
            vext = persist.tile([128, NST, 4, 65], BF16, tag="vext")
            nc.vector.memset(vext[:, :, :, 64:65], 1.0)

            # ---- startup DMA schedule ----
            wk_r = wk[:].rearrange("(kt p) e -> p kt e", p=128)
            wq_r = wq[:].rearrange("(kt p) e -> p kt e", p=128)
            wv_r = wv[:].rearrange("(kt p) e -> p kt e", p=128)
            wo_r = wo[:].rearrange("(pt p) n -> p pt n", p=128)
            ct_r = ct[:].rearrange("(mt p) s -> p mt s", p=128)
            st_r = st[:].rearrange("(mt p) s -> p mt s", p=128)

            def load_x_chunk(x_dram, c, dma_eng, split=None):
                """split: list of (engine, kt list) or None for single DMA."""
                xt = xtp.tile([128, 8, CH], BF16, tag="xt")
                xsrc = x_dram[:].rearrange("(kt p) s -> p kt s", p=128)
                if split is None:
                    dma_eng.dma_start(out=xt[:], in_=xsrc[:, :, c * CH:(c + 1) * CH])
                else:
                    for eng, kts in split:
                        for kt in kts:
                            eng.dma_start(out=xt[:, kt, :],
                                          in_=xsrc[:, kt, c * CH:(c + 1) * CH])
                return xt

            pre = {}
            # sync queue: wk, xk(kt0-3), wq, xq(kt0-3), rope tables chunk0
            nc.sync.dma_start(out=wk_t[:], in_=wk_r)
            # gpsimd queue: xk(kt4-7), xq(kt4-7), rt, tri, wv, xv, rest
            both = [(nc.sync, range(0, 4)), (nc.gpsimd, range(4, 8))]
            pre[(0, 'k')] = load_x_chunk(xk, 0, None, split=both)
            nc.sync.dma_start(out=wq_t[:], in_=wq_r)
            pre[(0, 'q')] = load_x_chunk(xq, 0, None, split=both)
            nc.sync.dma_start(out=ct_t[:, :, 0:CH], in_=ct_r[:, :, 0:CH])
            nc.sync.dma_start(out=st_t[:, :, 0:CH], in_=st_r[:, :, 0:CH])
            nc.gpsimd.dma_start(out=rt_t[:], in_=rt[:])
            nc.gpsimd.dma_start(out=tri_t[:], in_=tri[:])
            nc.gpsimd.dma_start(out=wv_t[:], in_=wv_r)
            pre[(0, 'v')] = load_x_chunk(xv, 0, nc.gpsimd)
            nc.gpsimd.dma_start(out=wo_t[:], in_=wo_r)
            nc.gpsimd.dma_start(out=ct_t[:, :, CH:S], in_=ct_r[:, :, CH:S])
            nc.gpsimd.dma_start(out=st_t[:, :, CH:S], in_=st_r[:, :, CH:S])

            def proj_rope_mms(xt, w_t, c, m):
                """Emit the 8 accumulating proj matmuls for (chunk c, m), plus
                the PSUM->SBUF eviction.  Returns the bf16 copy."""
                ps = projps.tile([128, CH], F32, tag="ps")
                for kt in range(8):
                    nc.tensor.matmul(
                        ps[:], lhsT=w_t[:, kt, m * 128:(m + 1) * 128],
                        rhs=xt[:, kt, :], start=(kt == 0), stop=(kt == 7),
                    )
                raw = rawp.tile([128, CH], BF16, tag="raw")
                nc.vector.tensor_copy(raw[:], ps[:])
                return raw

            def rope_tail(raw, dest, c, m):
                rps = projps.tile([128, CH], F32, tag="ps")
                nc.tensor.matmul(rps[:], lhsT=rt_t[:], rhs=raw[:], start=True, stop=True)
                t1 = rtp.tile([128, CH], BF16, tag="rtmp")
                nc.vector.tensor_mul(t1[:], rps[:], st_t[:, m, c * CH:(c + 1) * CH])
                t2 = rtp.tile([128, CH], BF16, tag="rtmp")
                nc.vector.tensor_mul(t2[:], raw[:], ct_t[:, m, c * CH:(c + 1) * CH])
                nc.vector.tensor_add(dest[:, m, c * CH:(c + 1) * CH], t1[:], t2[:])

            def vproj_stile(xt_v, stl):
                """Project s-tile stl of v into vext [s, (h, e)] layout."""
                ps = projps.tile([128, E], F32, tag="ps")
                for kt in range(8):
                    nc.tensor.matmul(
                        ps[:], lhsT=xt_v[:, kt, (stl % 4) * 128:(stl % 4) * 128 + 128],
                        rhs=wv_t[:, kt, :], start=(kt == 0), stop=(kt == 7),
                    )
                nc.vector.tensor_copy(
                    vext[:, stl, :, 0:64],
                    ps[:].rearrange("p (h e) -> p h e", h=4),
                )

            def wo_group(stl, n, eng=None):
                ps = projps.tile([128, CH], F32, tag="ps")
                for pair in range(2):
                    nc.tensor.matmul(
                        ps[:], lhsT=aT[:, pair, stl * 128:(stl + 1) * 128],
                        rhs=wo_t[:, pair, n * CH:(n + 1) * CH],
                        start=(pair == 0), stop=(pair == 1),
                    )
                ob = obufp.tile([128, CH], BF16, tag="ob")
                nc.vector.tensor_copy(ob[:], ps[:])
                (eng or nc.sync).dma_start(
                    out=out[:].rearrange("(t p) n -> p t n", p=128)[:, stl, n * CH:(n + 1) * CH],
                    in_=ob[:],
                )

            with (
                tc.tile_pool(name="spA", bufs=1, space="PSUM") as spA,
                tc.tile_pool(name="spB", bufs=1, space="PSUM") as spB,
                tc.tile_pool(name="opsum", bufs=2, space="PSUM") as opsum,
            ):
                # PE warm-up: dummy matmuls on a memset tile (no DMA dep)
                wps = spA.tile([128, 2 * CH], F32, tag="spa")
                for wi in range(20):
                    nc.tensor.matmul(
                        wps[:, 0:128], lhsT=warm_t[:], rhs=warm_t[:],
                        start=True, stop=True, skip_group_check=True,
                    )

                def attention_pair(c, pair, o_lo, o_hi, filler):
                    nj = 4 * c + 4

                    def emit_scores(jj):
                        """All four score MMs of a jj step, interleaved by
                        (dj, half) so the 64-row PE tiles 0/64 run pairwise
                        concurrent."""
                        sp0 = spA.tile([128, 2 * CH], F32, tag="spa")
                        sp1 = spB.tile([128, 2 * CH], F32, tag="spb")
                        for dj in range(2):
                            j = jj + dj
                            g = max(0, (j - 4 * c) * 128)
                            for half, sp in ((0, sp0), (64, sp1)):
                                nc.tensor.matmul(
                                    sp[:, dj * CH + g:(dj + 1) * CH],
                                    lhsT=kT[half:half + 64, pair, j * 128:(j + 1) * 128],
                                    rhs=qT[half:half + 64, pair,
                                           c * CH + g:(c + 1) * CH],
                                    start=True, stop=True,
                                )
                        return sp0, sp1

                    def emit_exp(jj, sp0, sp1):
                        gmin = max(0, (jj - 4 * c) * 128)
                        p0 = pblk.tile([128, 2 * CH], BF16, tag="p")
                        p1 = pblk.tile([128, 2 * CH], BF16, tag="p")
                        nc.scalar.activation(
                            p0[:, gmin:], sp0[:, gmin:],
                            mybir.ActivationFunctionType.Exp)
                        nc.scalar.activation(
                            p1[:, gmin:], sp1[:, gmin:],
                            mybir.ActivationFunctionType.Exp)
                        # causal mask on diagonal 128-col windows
                        for dj in range(2):
                            j = jj + dj
                            if j >= 4 * c:
                                g = (j - 4 * c) * 128
                                w0 = dj * CH + g
                                nc.vector.tensor_mul(
                                    p0[:, w0:w0 + 128], p0[:, w0:w0 + 128],
                                    tri_t[:])
                                nc.vector.tensor_mul(
                                    p1[:, w0:w0 + 128], p1[:, w0:w0 + 128],
                                    tri_t[:])
                        return p0, p1

                    def emit_av(jj, p0, p1):
                        for dj in range(2):
                            j = jj + dj
                            g = max(0, (j - 4 * c) * 128)
                            for p, ob, pt in ((p0, o_lo, 0), (p1, o_hi, 1)):
                                nc.tensor.matmul(
                                    ob[:, g:], lhsT=vext[:, j, 2 * pair + pt, :],
                                    rhs=p[:, dj * CH + g:(dj + 1) * CH],
                                    start=(j == 0), stop=(j == nj - 1),
                                    skip_group_check=True,
                                )

                    pend = None
                    for jj in range(0, nj, 2):
                        sp0, sp1 = emit_scores(jj)
                        if pend is not None:
                            emit_av(*pend)
                        else:
                            filler(1)
                        ps_ = emit_exp(jj, sp0, sp1)
                        pend = (jj, *ps_)
                        filler(1)
                    emit_av(*pend)
                    # evict O to SBUF (frees PSUM accumulators)
                    oc_lo = normp.tile([65, CH], F32, tag="oc")
                    nc.vector.tensor_copy(oc_lo[:], o_lo[:])
                    oc_hi = normp.tile([65, CH], F32, tag="oc")
                    nc.vector.tensor_copy(oc_hi[:], o_hi[:])
                    return oc_lo, oc_hi

                def norm_pair_a(c, pair, oc_lo, oc_hi, eng=None):
                    """DMA-gather denominators -> squat reciprocal -> scatter.
                    No PE work; emit before PE filler blocks."""
                    eng = eng or nc.gpsimd
                    den_g = normp.tile([128, 8], F32, tag="den")
                    eng.dma_start(out=den_g[:, 0:4], in_=oc_lo[64:65, :])
                    eng.dma_start(out=den_g[:, 4:8], in_=oc_hi[64:65, :])
                    rec_g = normp.tile([128, 8], F32, tag="rec")
                    nc.vector.reciprocal(rec_g[:], den_g[:])
                    rr_lo = normp.tile([1, CH], F32, tag="rr")
                    rr_hi = normp.tile([1, CH], F32, tag="rr")
                    eng.dma_start(out=rr_lo[:], in_=rec_g[:, 0:4])
                    eng.dma_start(out=rr_hi[:], in_=rec_g[:, 4:8])
                    return rr_lo, rr_hi

                def norm_tail_act(c, oc_lo, oc_hi):
                    """Tail-only variant: reciprocal via exp(-log(den)) on the
                    scalar engine (idle at the tail), avoiding the DMA
                    round-trip latency on the critical path."""
                    rr_lo = normp.tile([1, CH], F32, tag="rr")
                    rr_hi = normp.tile([1, CH], F32, tag="rr")
                    for oc, rr in ((oc_lo, rr_lo), (oc_hi, rr_hi)):
                        ln = normp.tile([1, CH], F32, tag="rr")
                        nc.scalar.activation(
                            ln[:], oc[64:65, :],
                            mybir.ActivationFunctionType.Ln)
                        nc.scalar.activation(
                            rr[:], ln[:],
                            mybir.ActivationFunctionType.Exp, scale=-1.0)
                    return rr_lo, rr_hi

                def norm_pair_b(c, pair, oc_lo, oc_hi, rr_lo, rr_hi, rnd=True):
                    """Broadcast reciprocal (K=1 matmul) and scale O into aT.
                    fp32r operands keep the matmul single-pass (plain fp32
                    lowers to a slow LOW/HIGH two-pass mode); ACT-produced
                    inputs aren't fp32r-rounded, so the tail uses plain fp32."""
                    for half, oc, rr in ((0, oc_lo, rr_lo), (64, oc_hi, rr_hi)):
                        bc = projps.tile([64, CH], F32, tag="ps")
                        if rnd:
                            nc.tensor.matmul(bc[:],
                                             lhsT=ones_t[:].bitcast(mybir.dt.float32r),
                                             rhs=rr[:].bitcast(mybir.dt.float32r),
                                             start=True, stop=True)
                        else:
                            nc.tensor.matmul(bc[:], lhsT=ones_t[:], rhs=rr[:],
                                             start=True, stop=True)
                        if half == 0:
                            nc.vector.tensor_mul(
                                aT[0:64, pair, c * CH:(c + 1) * CH], oc[0:64, :], bc[:])
                        else:
                            t64 = normp.tile([64, CH], BF16, tag="t64")
                            nc.vector.tensor_mul(t64[:], oc[0:64, :], bc[:])
                            nc.gpsimd.dma_start(
                                out=aT[64:128, pair, c * CH:(c + 1) * CH], in_=t64[:])

                # ---------- filler machinery ----------
                fill_items = []

                def filler(k):
                    for _ in range(k):
                        if fill_items:
                            fill_items.pop(0)()
                        else:
                            # keep-warm: starving the PE here lets the HAM
                            # clock gate re-throttle the array to 1.2 GHz for
                            # the rest of the phase, which costs far more
                            # than these dummy matmuls.
                            wt = projps.tile([128, 128], F32, tag="ps")
                            for _ in range(4):
                                nc.tensor.matmul(
                                    wt[:], lhsT=warm_t[:], rhs=warm_t[:],
                                    start=True, stop=True,
                                    skip_group_check=True)

                def flush_filler():
                    while fill_items:
                        fill_items.pop(0)()

                def add_proj_items(c):
                    """proj+rope for chunk c (q,k) and vproj; assumes x tiles
                    in pre[].  MM groups (A) and rope tails (B) are staggered
                    so a rope's PE matmul never queues right behind the DVE
                    eviction it depends on."""
                    state = {}
                    specs = [('k', wk_t, kT, 0), ('q', wq_t, qT, 0),
                             ('k', wk_t, kT, 1), ('q', wq_t, qT, 1)]

                    def a_item(i):
                        t, w_t, dest, m = specs[i]
                        state[i] = proj_rope_mms(pre[(c, t)], w_t, c, m)

                    def b_item(i):
                        t, w_t, dest, m = specs[i]
                        rope_tail(state.pop(i), dest, c, m)

                    order = [(a_item, 0), (a_item, 1), (b_item, 0), (a_item, 2),
                             (b_item, 1), (a_item, 3), (b_item, 2), (b_item, 3)]
                    for fn, i in order:
                        fill_items.append(lambda fn=fn, i=i: fn(i))
                    for stl in range(4 * c, 4 * c + 4):
                        def vitem(stl=stl, c=c):
                            vproj_stile(pre[(c, 'v')], stl)
                        fill_items.append(vitem)

                def add_wo_items(c):
                    for stl in range(4 * c, 4 * c + 4):
                        for n in range(2):
                            def witem(stl=stl, n=n):
                                wo_group(stl, n)
                            fill_items.append(witem)

                # ---------- chunk 0 projections (startup) ----------
                with nc.named_scope("proj_c0"):
                    add_proj_items(0)
                    flush_filler()

                # ---------- main chunk loop ----------
                for c in range(NCH):
                    # prefetch next chunk's activations
                    if c + 1 < NCH:
                        pre[(c + 1, 'k')] = load_x_chunk(xk, c + 1, nc.sync)
                        pre[(c + 1, 'q')] = load_x_chunk(xq, c + 1, nc.gpsimd)
                        pre[(c + 1, 'v')] = load_x_chunk(xv, c + 1, nc.sync)
                        add_proj_items(c + 1)
                    if c >= 1:
                        add_wo_items(c - 1)

                    for pair in range(2):
                        with nc.named_scope(f"att_c{c}_p{pair}"):
                            o_lo = opsum.tile([65, CH], F32, tag="o")
                            o_hi = opsum.tile([65, CH], F32, tag="o")
                            oc_lo, oc_hi = attention_pair(c, pair, o_lo, o_hi, filler)
                        with nc.named_scope(f"norm_c{c}_p{pair}"):
                            tail = (c == NCH - 1 and pair == 1)
                            if tail:
                                rr = norm_tail_act(c, oc_lo, oc_hi)
                            else:
                                rr = norm_pair_a(c, pair, oc_lo, oc_hi)
                            if not tail:
                                # defer the PE-side broadcast+scale into the
                                # filler stream so its DMA-chain wait overlaps
                                # subsequent attention instead of stalling the
                                # PE queue at the pair/chunk boundary.  For
                                # pair 1, flush proj/wo leftovers first (that
                                # PE work hides the chain latency), then park
                                # the broadcast for the next chunk's filler.
                                def nitem(c=c, pair=pair, oc_lo=oc_lo,
                                          oc_hi=oc_hi, rr=rr):
                                    norm_pair_b(c, pair, oc_lo, oc_hi, *rr)
                                if pair == 1:
                                    flush_filler()
                                    fill_items.insert(0, nitem)
                                else:
                                    fill_items.insert(min(2, len(fill_items)), nitem)
                            else:
                                flush_filler()
                                norm_pair_b(c, 1, oc_lo, oc_hi, *rr, rnd=False)

                with nc.named_scope("wo_c3"):
                    # final chunk's stores split across both DMA queues so the
                    # end-of-kernel drain halves
                    for stl in range(4 * (NCH - 1), 4 * NCH):
                        for n in range(2):
                            wo_group(stl, n,
                                     eng=(nc.sync if n == 0 else nc.gpsimd))

    nc.compile()
    return nc


def _host_tables():
    inv_freq = 1.0 / (10000.0 ** (np.arange(0, DK, 2, dtype=np.float64) / DK))
    pos = np.arange(S, dtype=np.float64)
    fr = pos[:, None] * inv_freq[None, :]          # [S, 32]
    sc8 = 1.0 / math.sqrt(math.sqrt(DK))           # fold 1/sqrt(DK) as sqrt into q and k
    cosT = (np.cos(fr).T * sc8).astype(np.float32)  # [32, S]
    sinT = (np.sin(fr).T * sc8).astype(np.float32)
    C = np.zeros((E, S), np.float32)
    Sg = np.zeros((E, S), np.float32)
    for hh in range(4):
        C[hh * 64:hh * 64 + 32] = cosT
        C[hh * 64 + 32:hh * 64 + 64] = cosT
        Sg[hh * 64:hh * 64 + 32] = -sinT
        Sg[hh * 64 + 32:hh * 64 + 64] = sinT
    # half-swap permutation for two stacked heads (128 rows)
    R = np.zeros((128, 128), np.float32)
    for hh in range(2):
        for j in range(32):
            R[hh * 64 + j, hh * 64 + 32 + j] = 1.0
            R[hh * 64 + 32 + j, hh * 64 + j] = 1.0
    # TRI[p, y] = 1 iff y >= p (causal window within a diagonal 128-tile)
    y = np.arange(128)[None, :]
    p = np.arange(128)[:, None]
    TRI = (y >= p).astype(np.float32)
    return C, Sg, R, TRI


def _build_in_maps(q, k, v, Wq, Wk, Wv, Wo):
    C, Sg, R, TRI = _host_tables()
    bf = ml_dtypes.bfloat16
    in_maps = []
    for c in range(8):
        b = c // 4
        h0 = 4 * (c % 4)
        sl = slice(h0 * DK, (h0 + 4) * DK)
        in_maps.append({
            "xq": np.ascontiguousarray(q[b].T).astype(bf),
            "xk": np.ascontiguousarray(k[b].T).astype(bf),
            "xv": np.ascontiguousarray(v[b].T).astype(bf),
            "wq": np.ascontiguousarray(Wq[sl].T).astype(bf),
            "wk": np.ascontiguousarray(Wk[sl].T).astype(bf),
            "wv": np.ascontiguousarray(Wv[sl].T).astype(bf),
            "wo": np.ascontiguousarray(Wo[:, sl].T).astype(bf),
            "ct": C.astype(bf),
            "st": Sg.astype(bf),
            "rt": R.astype(bf),
            "tri": TRI.astype(bf),
        })
    return in_maps


_program_cache = {}


def kernel(q, k, v, mask, Wq, bq, Wk, bk, Wv, bv, Wo, bo):
    q = np.asarray(q, np.float32)
    k = np.asarray(k, np.float32)
    v = np.asarray(v, np.float32)
    mask = np.asarray(mask)
    Wq, bq = np.asarray(Wq, np.float32), np.asarray(bq, np.float32)
    Wk, bk = np.asarray(Wk, np.float32), np.asarray(bk, np.float32)
    Wv, bv = np.asarray(Wv, np.float32), np.asarray(bv, np.float32)
    Wo, bo = np.asarray(Wo, np.float32), np.asarray(bo, np.float32)

    causal = np.array_equal(
        np.asarray(mask[0, 0], np.int64), np.tril(np.ones((S, S), np.int64)))
    if not causal or np.any(bq) or np.any(bk):
        return _np_reference_fallback(q, k, v, mask, Wq, bq, Wk, bk, Wv, bv, Wo, bo)

    if "nc" not in _program_cache:
        _program_cache["nc"] = _build_program()
    nc = _program_cache["nc"]

    in_maps = _build_in_maps(q, k, v, Wq, Wk, Wv, Wo)
    res = run_bass_kernel_spmd(nc, in_maps, core_ids=list(range(8)))

    out = np.zeros((B, S, D), np.float32)
    for c in range(8):
        out[c // 4] += np.asarray(res.results[c]["out"], np.float32)
    # bv folds through softmax (rows sum to 1) and Wo; bo direct.
    out += (bv @ Wo.T + bo)[None, None, :]
    return out


# revision 36
# speedup vs baseline: 1.0694x; 1.0694x over previous
"""Multi-head attention (RoPE, causal) Trainium2 Bass kernel, 8-way sharded.

Sharding: core c handles batch b = c//4 and heads 4*(c%4)..4*(c%4)+3
(B*H = 32 head-rows -> 4 per core).  QKV/out projections are
Megatron-sliced per core; per-core partial outputs (row-parallel Wo)
are summed on the host.

v2 schedule: the attention inner loop is software-pipelined (scores of
step jj issue before the AV of step jj-2, so the PE never queues behind
the scalar-engine exp), and the projections / Wo / V-projection of
neighbouring chunks are drip-fed as PE filler inside the exp-bound
attention phase.  Normalization gathers softmax denominators via
SBUF->SBUF DMA into a squat [128,8] tile (parallel-lane reciprocal),
broadcasts the reciprocal back with a K=1 matmul.  Output is stored
bf16 and summed on the host in fp32.

Problem constants (hardcoded per contract):
  B=2, S=2048, D=1024, H=16, DK=64
"""

import math

import ml_dtypes
import numpy as np

import concourse.bass as bass
import concourse.mybir as mybir
import concourse.tile as tile
from concourse import bacc
from concourse.bass_utils import run_bass_kernel_spmd

B, S, D, H, DK = 2, 2048, 1024, 16, 64
E = 256            # head dims per core (4 heads x 64)
CH = 512           # sequence chunk (matmul free dim)
NCH = S // CH      # 4
NST = S // 128     # 16 s-tiles
BF16 = mybir.dt.bfloat16
F32 = mybir.dt.float32


def _np_reference_fallback(q, k, v, mask, Wq, bq, Wk, bk, Wv, bv, Wo, bo):
    """Pure-numpy reference path (only used for inputs outside the
    contract: non-causal mask or nonzero qkv biases)."""
    qh = (q @ Wq.T + bq).reshape(B, S, H, DK)
    kh = (k @ Wk.T + bk).reshape(B, S, H, DK)
    vh = (v @ Wv.T + bv).reshape(B, S, H, DK)
    inv_freq = 1.0 / (10000.0 ** (np.arange(0, DK, 2, dtype=np.float32) / DK))
    pos = np.arange(S, dtype=np.float32)
    fr = pos[:, None] * inv_freq[None, :]
    cos, sin = np.cos(fr)[:, None, :], np.sin(fr)[:, None, :]

    def rope(x):
        t = DK // 2
        x1, x2 = x[..., :t], x[..., t:]
        return np.concatenate([x1 * cos - x2 * sin, x1 * sin + x2 * cos], -1)

    qh, kh = rope(qh), rope(kh)
    sc = np.einsum('bqhd,bkhd->bhqk', qh, kh) / math.sqrt(DK)
    sc = np.where(mask == 0, np.float32(-10000.0), sc)
    sc = sc - sc.max(-1, keepdims=True)
    e = np.exp(sc)
    attn = e / e.sum(-1, keepdims=True)
    out = np.einsum('bhqk,bkhd->bqhd', attn, vh).reshape(B, S, D)
    return (out @ Wo.T + bo).astype(np.float32)


def _build_program():
    nc = bacc.Bacc(None, target_bir_lowering=False)

    dp = nc.declare_dram_parameter
    xq = dp("xq", [D, S], BF16, isOutput=False)   # q[b].T
    xk = dp("xk", [D, S], BF16, isOutput=False)
    xv = dp("xv", [D, S], BF16, isOutput=False)
    wq = dp("wq", [D, E], BF16, isOutput=False)   # Wq_c.T
    wk = dp("wk", [D, E], BF16, isOutput=False)
    wv = dp("wv", [D, E], BF16, isOutput=False)
    wo = dp("wo", [E, D], BF16, isOutput=False)   # Wo_c.T rows
    ct = dp("ct", [E, S], BF16, isOutput=False)   # cos table (1/sqrt8 folded)
    st = dp("st", [E, S], BF16, isOutput=False)   # signed sin table
    rt = dp("rt", [128, 128], BF16, isOutput=False)  # half-swap permutation
    tri = dp("tri", [128, 128], BF16, isOutput=False)  # causal 0/1 triangle
    out = dp("out", [S, D], BF16, isOutput=True)

    with tile.TileContext(nc) as tc:
        with (
            tc.tile_pool(name="const", bufs=1) as const,
            tc.tile_pool(name="persist", bufs=1) as persist,
            tc.tile_pool(name="xt", bufs=6) as xtp,
            tc.tile_pool(name="raw", bufs=4) as rawp,
            tc.tile_pool(name="ropetmp", bufs=4) as rtp,
            tc.tile_pool(name="pblk", bufs=4) as pblk,
            tc.tile_pool(name="norm", bufs=8) as normp,
            tc.tile_pool(name="obuf", bufs=4) as obufp,
            tc.tile_pool(name="projps", bufs=2, space="PSUM") as projps,
        ):
            # ---- constants to SBUF ----
            warm_t = const.tile([128, 128], BF16, tag="warm")
            nc.vector.memset(warm_t[:], 0.25)
            rt_t = const.tile([128, 128], BF16, tag="rt")
            wk_t = const.tile([128, 8, E], BF16, tag="wk")
            wq_t = const.tile([128, 8, E], BF16, tag="wq")
            ct_t = const.tile([128, 2, S], BF16, tag="ct")
            st_t = const.tile([128, 2, S], BF16, tag="st")
            wv_t = const.tile([128, 8, E], BF16, tag="wv")
            tri_t = const.tile([128, 128], BF16, tag="tri")
            wo_t = const.tile([128, 2, D], BF16, tag="wo")
            ones_t = const.tile([1, 64], F32, tag="ones")
            nc.vector.memset(ones_t[:], 1.0)

            # ---- persistent intermediates ----
            qT = persist.tile([128, 2, S], BF16, tag="qT")   # partitions: e%128, dim1: e//128
            kT = persist.tile([128, 2, S], BF16, tag="kT")
            aT = persist.tile([128, 2, S], BF16, tag="aT")
            vext = persist.tile([128, NST, 4, 65], BF16, tag="vext")
            nc.vector.memset(vext[:, :, :, 64:65], 1.0)

            # ---- startup DMA schedule ----
            wk_r = wk[:].rearrange("(kt p) e -> p kt e", p=128)
            wq_r = wq[:].rearrange("(kt p) e -> p kt e", p=128)
            wv_r = wv[:].rearrange("(kt p) e -> p kt e", p=128)
            wo_r = wo[:].rearrange("(pt p) n -> p pt n", p=128)
            ct_r = ct[:].rearrange("(mt p) s -> p mt s", p=128)
            st_r = st[:].rearrange("(mt p) s -> p mt s", p=128)

            def load_x_chunk(x_dram, c, dma_eng, split=None):
                """split: list of (engine, kt list) or None for single DMA."""
                xt = xtp.tile([128, 8, CH], BF16, tag="xt")
                xsrc = x_dram[:].rearrange("(kt p) s -> p kt s", p=128)
                if split is None:
                    dma_eng.dma_start(out=xt[:], in_=xsrc[:, :, c * CH:(c + 1) * CH])
                else:
                    for eng, kts in split:
                        for kt in kts:
                            eng.dma_start(out=xt[:, kt, :],
                                          in_=xsrc[:, kt, c * CH:(c + 1) * CH])
                return xt

            pre = {}
            # sync queue: wk, xk(kt0-3), wq, xq(kt0-3), rope tables chunk0
            nc.sync.dma_start(out=wk_t[:], in_=wk_r)
            # gpsimd queue: xk(kt4-7), xq(kt4-7), rt, tri, wv, xv, rest
            both = [(nc.sync, range(0, 4)), (nc.gpsimd, range(4, 8))]
            pre[(0, 'k')] = load_x_chunk(xk, 0, None, split=both)
            nc.sync.dma_start(out=wq_t[:], in_=wq_r)
            pre[(0, 'q')] = load_x_chunk(xq, 0, None, split=both)
            nc.sync.dma_start(out=ct_t[:, :, 0:CH], in_=ct_r[:, :, 0:CH])
            nc.sync.dma_start(out=st_t[:, :, 0:CH], in_=st_r[:, :, 0:CH])
            nc.gpsimd.dma_start(out=rt_t[:], in_=rt[:])
            nc.gpsimd.dma_start(out=tri_t[:], in_=tri[:])
            nc.gpsimd.dma_start(out=wv_t[:], in_=wv_r)
            pre[(0, 'v')] = load_x_chunk(xv, 0, nc.gpsimd)
            nc.gpsimd.dma_start(out=wo_t[:], in_=wo_r)
            nc.gpsimd.dma_start(out=ct_t[:, :, CH:S], in_=ct_r[:, :, CH:S])
            nc.gpsimd.dma_start(out=st_t[:, :, CH:S], in_=st_r[:, :, CH:S])

            def proj_rope_mms(xt, w_t, c, m):
                """Emit the 8 accumulating proj matmuls for (chunk c, m), plus
                the PSUM->SBUF eviction.  Returns the bf16 copy."""
                ps = projps.tile([128, CH], F32, tag="ps")
                for kt in range(8):
                    nc.tensor.matmul(
                        ps[:], lhsT=w_t[:, kt, m * 128:(m + 1) * 128],
                        rhs=xt[:, kt, :], start=(kt == 0), stop=(kt == 7),
                    )
                raw = rawp.tile([128, CH], BF16, tag="raw")
                nc.vector.tensor_copy(raw[:], ps[:])
                return raw

            def rope_tail(raw, dest, c, m):
                rps = projps.tile([128, CH], F32, tag="ps")
                nc.tensor.matmul(rps[:], lhsT=rt_t[:], rhs=raw[:], start=True, stop=True)
                t1 = rtp.tile([128, CH], BF16, tag="rtmp")
                nc.vector.tensor_mul(t1[:], rps[:], st_t[:, m, c * CH:(c + 1) * CH])
                t2 = rtp.tile([128, CH], BF16, tag="rtmp")
                nc.vector.tensor_mul(t2[:], raw[:], ct_t[:, m, c * CH:(c + 1) * CH])
                nc.vector.tensor_add(dest[:, m, c * CH:(c + 1) * CH], t1[:], t2[:])

            def vproj_stile(xt_v, stl):
                """Project s-tile stl of v into vext [s, (h, e)] layout."""
                ps = projps.tile([128, E], F32, tag="ps")
                for kt in range(8):
                    nc.tensor.matmul(
                        ps[:], lhsT=xt_v[:, kt, (stl % 4) * 128:(stl % 4) * 128 + 128],
                        rhs=wv_t[:, kt, :], start=(kt == 0), stop=(kt == 7),
                    )
                nc.vector.tensor_copy(
                    vext[:, stl, :, 0:64],
                    ps[:].rearrange("p (h e) -> p h e", h=4),
                )

            def wo_group(stl, n):
                ps = projps.tile([128, CH], F32, tag="ps")
                for pair in range(2):
                    nc.tensor.matmul(
                        ps[:], lhsT=aT[:, pair, stl * 128:(stl + 1) * 128],
                        rhs=wo_t[:, pair, n * CH:(n + 1) * CH],
                        start=(pair == 0), stop=(pair == 1),
                    )
                ob = obufp.tile([128, CH], BF16, tag="ob")
                nc.vector.tensor_copy(ob[:], ps[:])
                nc.sync.dma_start(
                    out=out[:].rearrange("(t p) n -> p t n", p=128)[:, stl, n * CH:(n + 1) * CH],
                    in_=ob[:],
                )

            with (
                tc.tile_pool(name="spA", bufs=1, space="PSUM") as spA,
                tc.tile_pool(name="spB", bufs=1, space="PSUM") as spB,
                tc.tile_pool(name="opsum", bufs=2, space="PSUM") as opsum,
            ):
                # PE warm-up: dummy matmuls on a memset tile (no DMA dep)
                wps = spA.tile([128, 2 * CH], F32, tag="spa")
                for wi in range(20):
                    nc.tensor.matmul(
                        wps[:, 0:128], lhsT=warm_t[:], rhs=warm_t[:],
                        start=True, stop=True, skip_group_check=True,
                    )

                def attention_pair(c, pair, o_lo, o_hi, filler):
                    nj = 4 * c + 4

                    def emit_scores(jj):
                        """All four score MMs of a jj step, interleaved by
                        (dj, half) so the 64-row PE tiles 0/64 run pairwise
                        concurrent."""
                        sp0 = spA.tile([128, 2 * CH], F32, tag="spa")
                        sp1 = spB.tile([128, 2 * CH], F32, tag="spb")
                        for dj in range(2):
                            j = jj + dj
                            g = max(0, (j - 4 * c) * 128)
                            for half, sp in ((0, sp0), (64, sp1)):
                                nc.tensor.matmul(
                                    sp[:, dj * CH + g:(dj + 1) * CH],
                                    lhsT=kT[half:half + 64, pair, j * 128:(j + 1) * 128],
                                    rhs=qT[half:half + 64, pair,
                                           c * CH + g:(c + 1) * CH],
                                    start=True, stop=True,
                                )
                        return sp0, sp1

                    def emit_exp(jj, sp0, sp1):
                        gmin = max(0, (jj - 4 * c) * 128)
                        p0 = pblk.tile([128, 2 * CH], BF16, tag="p")
                        p1 = pblk.tile([128, 2 * CH], BF16, tag="p")
                        nc.scalar.activation(
                            p0[:, gmin:], sp0[:, gmin:],
                            mybir.ActivationFunctionType.Exp)
                        nc.scalar.activation(
                            p1[:, gmin:], sp1[:, gmin:],
                            mybir.ActivationFunctionType.Exp)
                        # causal mask on diagonal 128-col windows
                        for dj in range(2):
                            j = jj + dj
                            if j >= 4 * c:
                                g = (j - 4 * c) * 128
                                w0 = dj * CH + g
                                nc.vector.tensor_mul(
                                    p0[:, w0:w0 + 128], p0[:, w0:w0 + 128],
                                    tri_t[:])
                                nc.vector.tensor_mul(
                                    p1[:, w0:w0 + 128], p1[:, w0:w0 + 128],
                                    tri_t[:])
                        return p0, p1

                    def emit_av(jj, p0, p1):
                        for dj in range(2):
                            j = jj + dj
                            g = max(0, (j - 4 * c) * 128)
                            for p, ob, pt in ((p0, o_lo, 0), (p1, o_hi, 1)):
                                nc.tensor.matmul(
                                    ob[:, g:], lhsT=vext[:, j, 2 * pair + pt, :],
                                    rhs=p[:, dj * CH + g:(dj + 1) * CH],
                                    start=(j == 0), stop=(j == nj - 1),
                                    skip_group_check=True,
                                )

                    pend = None
                    for jj in range(0, nj, 2):
                        sp0, sp1 = emit_scores(jj)
                        if pend is not None:
                            emit_av(*pend)
                        else:
                            filler(1)
                        ps_ = emit_exp(jj, sp0, sp1)
                        pend = (jj, *ps_)
                        filler(1)
                    emit_av(*pend)
                    # evict O to SBUF (frees PSUM accumulators)
                    oc_lo = normp.tile([65, CH], F32, tag="oc")
                    nc.vector.tensor_copy(oc_lo[:], o_lo[:])
                    oc_hi = normp.tile([65, CH], F32, tag="oc")
                    nc.vector.tensor_copy(oc_hi[:], o_hi[:])
                    return oc_lo, oc_hi

                def norm_pair_a(c, pair, oc_lo, oc_hi, eng=None):
                    """DMA-gather denominators -> squat reciprocal -> scatter.
                    No PE work; emit before PE filler blocks."""
                    eng = eng or nc.gpsimd
                    den_g = normp.tile([128, 8], F32, tag="den")
                    eng.dma_start(out=den_g[:, 0:4], in_=oc_lo[64:65, :])
                    eng.dma_start(out=den_g[:, 4:8], in_=oc_hi[64:65, :])
                    rec_g = normp.tile([128, 8], F32, tag="rec")
                    nc.vector.reciprocal(rec_g[:], den_g[:])
                    rr_lo = normp.tile([1, CH], F32, tag="rr")
                    rr_hi = normp.tile([1, CH], F32, tag="rr")
                    eng.dma_start(out=rr_lo[:], in_=rec_g[:, 0:4])
                    eng.dma_start(out=rr_hi[:], in_=rec_g[:, 4:8])
                    return rr_lo, rr_hi

                def norm_pair_b(c, pair, oc_lo, oc_hi, rr_lo, rr_hi):
                    """Broadcast reciprocal (K=1 matmul) and scale O into aT.
                    fp32r operands keep the matmul single-pass (plain fp32
                    lowers to a slow LOW/HIGH two-pass mode)."""
                    for half, oc, rr in ((0, oc_lo, rr_lo), (64, oc_hi, rr_hi)):
                        bc = projps.tile([64, CH], F32, tag="ps")
                        nc.tensor.matmul(bc[:],
                                         lhsT=ones_t[:].bitcast(mybir.dt.float32r),
                                         rhs=rr[:].bitcast(mybir.dt.float32r),
                                         start=True, stop=True)
                        if half == 0:
                            nc.vector.tensor_mul(
                                aT[0:64, pair, c * CH:(c + 1) * CH], oc[0:64, :], bc[:])
                        else:
                            t64 = normp.tile([64, CH], BF16, tag="t64")
                            nc.vector.tensor_mul(t64[:], oc[0:64, :], bc[:])
                            nc.gpsimd.dma_start(
                                out=aT[64:128, pair, c * CH:(c + 1) * CH], in_=t64[:])

                # ---------- filler machinery ----------
                fill_items = []

                def filler(k):
                    for _ in range(min(k, len(fill_items))):
                        fill_items.pop(0)()

                def flush_filler():
                    while fill_items:
                        fill_items.pop(0)()

                def add_proj_items(c):
                    """proj+rope for chunk c (q,k) and vproj; assumes x tiles
                    in pre[].  MM groups (A) and rope tails (B) are staggered
                    so a rope's PE matmul never queues right behind the DVE
                    eviction it depends on."""
                    state = {}
                    specs = [('k', wk_t, kT, 0), ('q', wq_t, qT, 0),
                             ('k', wk_t, kT, 1), ('q', wq_t, qT, 1)]

                    def a_item(i):
                        t, w_t, dest, m = specs[i]
                        state[i] = proj_rope_mms(pre[(c, t)], w_t, c, m)

                    def b_item(i):
                        t, w_t, dest, m = specs[i]
                        rope_tail(state.pop(i), dest, c, m)

                    order = [(a_item, 0), (a_item, 1), (b_item, 0), (a_item, 2),
                             (b_item, 1), (a_item, 3), (b_item, 2), (b_item, 3)]
                    for fn, i in order:
                        fill_items.append(lambda fn=fn, i=i: fn(i))
                    for stl in range(4 * c, 4 * c + 4):
                        def vitem(stl=stl, c=c):
                            vproj_stile(pre[(c, 'v')], stl)
                        fill_items.append(vitem)

                def add_wo_items(c):
                    for stl in range(4 * c, 4 * c + 4):
                        for n in range(2):
                            def witem(stl=stl, n=n):
                                wo_group(stl, n)
                            fill_items.append(witem)

                # ---------- chunk 0 projections (startup) ----------
                with nc.named_scope("proj_c0"):
                    add_proj_items(0)
                    flush_filler()

                # ---------- main chunk loop ----------
                for c in range(NCH):
                    # prefetch next chunk's activations
                    if c + 1 < NCH:
                        pre[(c + 1, 'k')] = load_x_chunk(xk, c + 1, nc.sync)
                        pre[(c + 1, 'q')] = load_x_chunk(xq, c + 1, nc.gpsimd)
                        pre[(c + 1, 'v')] = load_x_chunk(xv, c + 1, nc.sync)
                        add_proj_items(c + 1)
                    if c >= 1:
                        add_wo_items(c - 1)

                    for pair in range(2):
                        with nc.named_scope(f"att_c{c}_p{pair}"):
                            o_lo = opsum.tile([65, CH], F32, tag="o")
                            o_hi = opsum.tile([65, CH], F32, tag="o")
                            oc_lo, oc_hi = attention_pair(c, pair, o_lo, o_hi, filler)
                        with nc.named_scope(f"norm_c{c}_p{pair}"):
                            tail = (c == NCH - 1 and pair == 1)
                            rr = norm_pair_a(c, pair, oc_lo, oc_hi,
                                             eng=nc.sync if tail else None)
                            if not tail:
                                # defer the PE-side broadcast+scale into the
                                # filler stream so its DMA-chain wait overlaps
                                # subsequent attention instead of stalling the
                                # PE queue at the pair/chunk boundary.  For
                                # pair 1, flush proj/wo leftovers first (that
                                # PE work hides the chain latency), then park
                                # the broadcast for the next chunk's filler.
                                def nitem(c=c, pair=pair, oc_lo=oc_lo,
                                          oc_hi=oc_hi, rr=rr):
                                    norm_pair_b(c, pair, oc_lo, oc_hi, *rr)
                                if pair == 1:
                                    flush_filler()
                                    fill_items.insert(0, nitem)
                                else:
                                    fill_items.insert(min(2, len(fill_items)), nitem)
                            else:
                                flush_filler()
                                norm_pair_b(c, 1, oc_lo, oc_hi, *rr)

                with nc.named_scope("wo_c3"):
                    add_wo_items(NCH - 1)
                    flush_filler()

    nc.compile()
    return nc


def _host_tables():
    inv_freq = 1.0 / (10000.0 ** (np.arange(0, DK, 2, dtype=np.float64) / DK))
    pos = np.arange(S, dtype=np.float64)
    fr = pos[:, None] * inv_freq[None, :]          # [S, 32]
    sc8 = 1.0 / math.sqrt(math.sqrt(DK))           # fold 1/sqrt(DK) as sqrt into q and k
    cosT = (np.cos(fr).T * sc8).astype(np.float32)  # [32, S]
    sinT = (np.sin(fr).T * sc8).astype(np.float32)
    C = np.zeros((E, S), np.float32)
    Sg = np.zeros((E, S), np.float32)
    for hh in range(4):
        C[hh * 64:hh * 64 + 32] = cosT
        C[hh * 64 + 32:hh * 64 + 64] = cosT
        Sg[hh * 64:hh * 64 + 32] = -sinT
        Sg[hh * 64 + 32:hh * 64 + 64] = sinT
    # half-swap permutation for two stacked heads (128 rows)
    R = np.zeros((128, 128), np.float32)
    for hh in range(2):
        for j in range(32):
            R[hh * 64 + j, hh * 64 + 32 + j] = 1.0
            R[hh * 64 + 32 + j, hh * 64 + j] = 1.0
    # TRI[p, y] = 1 iff y >= p (causal window within a diagonal 128-tile)
    y = np.arange(128)[None, :]
    p = np.arange(128)[:, None]
    TRI = (y >= p).astype(np.float32)
    return C, Sg, R, TRI


def _build_in_maps(q, k, v, Wq, Wk, Wv, Wo):
    C, Sg, R, TRI = _host_tables()
    bf = ml_dtypes.bfloat16
    in_maps = []
    for c in range(8):
        b = c // 4
        h0 = 4 * (c % 4)
        sl = slice(h0 * DK, (h0 + 4) * DK)
        in_maps.append({
            "xq": np.ascontiguousarray(q[b].T).astype(bf),
            "xk": np.ascontiguousarray(k[b].T).astype(bf),
            "xv": np.ascontiguousarray(v[b].T).astype(bf),
            "wq": np.ascontiguousarray(Wq[sl].T).astype(bf),
            "wk": np.ascontiguousarray(Wk[sl].T).astype(bf),
            "wv": np.ascontiguousarray(Wv[sl].T).astype(bf),
            "wo": np.ascontiguousarray(Wo[:, sl].T).astype(bf),
            "ct": C.astype(bf),
            "st": Sg.astype(bf),
            "rt": R.astype(bf),
            "tri": TRI.astype(bf),
        })
    return in_maps


_program_cache = {}


def kernel(q, k, v, mask, Wq, bq, Wk, bk, Wv, bv, Wo, bo):
    q = np.asarray(q, np.float32)
    k = np.asarray(k, np.float32)
    v = np.asarray(v, np.float32)
    mask = np.asarray(mask)
    Wq, bq = np.asarray(Wq, np.float32), np.asarray(bq, np.float32)
    Wk, bk = np.asarray(Wk, np.float32), np.asarray(bk, np.float32)
    Wv, bv = np.asarray(Wv, np.float32), np.asarray(bv, np.float32)
    Wo, bo = np.asarray(Wo, np.float32), np.asarray(bo, np.float32)

    causal = np.array_equal(
        np.asarray(mask[0, 0], np.int64), np.tril(np.ones((S, S), np.int64)))
    if not causal or np.any(bq) or np.any(bk):
        return _np_reference_fallback(q, k, v, mask, Wq, bq, Wk, bk, Wv, bv, Wo, bo)

    if "nc" not in _program_cache:
        _program_cache["nc"] = _build_program()
    nc = _program_cache["nc"]

    in_maps = _build_in_maps(q, k, v, Wq, Wk, Wv, Wo)
    res = run_bass_kernel_spmd(nc, in_maps, core_ids=list(range(8)))

    out = np.zeros((B, S, D), np.float32)
    for c in range(8):
        out[c // 4] += np.asarray(res.results[c]["out"], np.float32)
    # bv folds through softmax (rows sum to 1) and Wo; bo direct.
    out += (bv @ Wo.T + bo)[None, None, :]
    return out
